# revision 4
# baseline (speedup 1.0000x reference)
"""CSNet kernel for 8 Trainium2 NeuronCores.

Strategy (per sharding hint): pure data parallelism — batch 128 is split
16-per-core across the 8 cores; all weights are replicated (folded into
dense matrices host-side); each core runs the full per-sample pipeline:

  x (64,1000) --[spatial convs + merge conv + BN folded to one 32x64
  matmul]--> Xs (32,1000) --[grouped temporal convs, BN folded]--> Xt
  (192,1000) --> Gram (192,192) --> blocked Cholesky (basic ops only;
  the `cholesky` HLO is unsupported by neuronx-cc) --> log-diag +
  strict-lower-triangle contraction with masked FC weights --> (4,).

Everything after input sharding runs on-device via one SPMD executable
(jax.pmap over the 8 NeuronCores). All contractions use HIGHEST
precision so fp32 matches the fp32 reference.
"""

import numpy as np
import jax
import jax.numpy as jnp

B, N_CHANS, T = 128, 64, 1000
N_CORES = 8
SHARD = B // N_CORES  # 16
FEATURE_DIM = [(3, 4), (4, 8), (4, 16), (7, 32), (240, 64)]
SUM_SP = 1078
FILTERS = [41, 51, 61]
N_FEAT = 192
BN_EPS = 1e-5
PADMAX = 30
HP = jax.lax.Precision.HIGHEST
NB = 64  # cholesky panel width

_cache = {}


def _fold(inputs):
    """Fold spatial convs + merge conv + BN into (A, c); fold BN into
    temporal weights; build masked FC weights for the tangent map."""
    f32 = np.float32
    bn = 1.0 / np.sqrt(1.0 + BN_EPS)

    S = np.zeros((SUM_SP, N_CHANS), f32)
    bs = np.zeros((SUM_SP,), f32)
    r0 = 0
    for i, (d0, d1) in enumerate(FEATURE_DIM):
        sw = np.asarray(inputs[f"sw{i}"], f32)[:, 0, :, 0]  # (d0, d1)
        sb = np.asarray(inputs[f"sb{i}"], f32)
        H = N_CHANS - d1 + 1
        for o in range(d0):
            for h in range(H):
                S[r0 + o * H + h, h:h + d1] = sw[o]
                bs[r0 + o * H + h] = sb[o]
        r0 += d0 * H
    assert r0 == SUM_SP

    mw = np.asarray(inputs["mw"], f32)[:, 0, :, 0]          # (32, 1078)
    mb = np.asarray(inputs["mb"], f32)
    mg = np.asarray(inputs["mg"], f32) * bn
    mbt = np.asarray(inputs["mbt"], f32)
    A = (mg[:, None] * (mw @ S))                             # (32, 64)
    c = mg * (mw @ bs + mb) + mbt                            # (32,)

    tws, tbs = [], []
    for i in range(len(FILTERS)):
        tw = np.asarray(inputs[f"tw{i}"], f32)[:, 0, 0, :]   # (64, S)
        tb = np.asarray(inputs[f"tb{i}"], f32)
        tg = np.asarray(inputs[f"tg{i}"], f32) * bn
        tbt = np.asarray(inputs[f"tbt{i}"], f32)
        tws.append(tg[:, None] * tw)
        tbs.append(tg * tb + tbt)

    fcw = np.asarray(inputs["fcw"], f32)                     # (4, 18528)
    fcb = np.asarray(inputs["fcb"], f32)
    rows, cols = np.tril_indices(N_FEAT, -1)                 # row-major
    Wdiag = fcw[:, :N_FEAT].copy()                           # (4, 192)
    Wtril = np.zeros((fcw.shape[0], N_FEAT, N_FEAT), f32)
    Wtril[:, rows, cols] = fcw[:, N_FEAT:]

    masks = np.zeros((N_FEAT, N_FEAT), f32)                  # mask_ge[gj, i]
    for gj in range(N_FEAT):
        masks[gj, gj:] = 1.0

    return A, c, tws, tbs, Wdiag, Wtril, fcb, masks


def _build(inputs):
    A, c, tws, tbs, Wdiag, Wtril, fcb, masks = _fold(inputs)
    eye = np.eye(N_FEAT, dtype=np.float32)

    def shard_fn(x):  # x: (SHARD, 64, 1000)
        Xs = jnp.einsum('mc,bct->bmt', A, x, precision=HP) + c[None, :, None]
        Xsi = Xs[:, :, None, :]                               # (b,32,1,T)
        Xt = []
        for f, size in enumerate(FILTERS):
            P = size // 2
            w4 = tws[f][:, None, None, :]                     # (64,1,1,S)
            y = jax.lax.conv_general_dilated(
                Xsi, w4, (1, 1), ((0, 0), (P, P)),
                dimension_numbers=('NCHW', 'OIHW', 'NCHW'),
                feature_group_count=32, precision=HP)
            Xt.append(y[:, :, 0, :] + tbs[f][None, :, None])
        Xt = jnp.concatenate(Xt, 1)                           # (SHARD,192,T)
        G = jnp.einsum('bct,bdt->bcd', Xt, Xt, precision=HP) / (T - 1)

        # blocked Cholesky (lower), panels of NB columns, rank-2 steps
        Aw = G
        L = jnp.zeros_like(G)
        nblk = N_FEAT // NB
        for jb in range(nblk):
            j0 = jb * NB
            Pn = Aw[:, :, j0:j0 + NB]                         # (b,192,NB)
            colsl = []
            for j in range(0, NB, 2):
                gj = j0 + j
                d0 = jnp.sqrt(Pn[:, gj, j])
                col0 = (Pn[:, :, j] * (1.0 / d0)[:, None]
                        * masks[gj][None, :])
                c1 = Pn[:, :, j + 1] - col0 * col0[:, gj + 1][:, None]
                d1 = jnp.sqrt(c1[:, gj + 1])
                col1 = (c1 * (1.0 / d1)[:, None]
                        * masks[gj + 1][None, :])
                colsl += [col0, col1]
                CP = jnp.stack([col0, col1], -1)              # (b,192,2)
                R = jnp.stack([col0[:, j0:j0 + NB],
                               col1[:, j0:j0 + NB]], 1)       # (b,2,NB)
                Pn = Pn - jnp.einsum('bik,bkj->bij', CP, R, precision=HP)
            Lblk = jnp.stack(colsl, -1)                       # (b,192,NB)
            L = L.at[:, :, j0:j0 + NB].set(Lblk)
            if jb + 1 < nblk:
                upd = jnp.einsum('bik,bjk->bij', Lblk,
                                 Lblk[:, j0 + NB:, :], precision=HP)
                Aw = Aw.at[:, :, j0 + NB:].add(-upd)

        diag = jnp.sum(L * eye, -1)                           # (b,192)
        out = (jnp.log(diag) @ Wdiag.T
               + jnp.einsum('bij,kij->bk', L, Wtril, precision=HP)
               + fcb[None, :])
        return out

    return jax.pmap(shard_fn)


def kernel(**inputs):
    key = 'fn'
    if key not in _cache:
        _cache[key] = _build(inputs)
    fn = _cache[key]
    x = np.asarray(inputs["x"], np.float32).reshape(N_CORES, SHARD,
                                                    N_CHANS, T)
    out = fn(x)                                               # (8,16,4)
    return np.asarray(out).reshape(B, -1).astype(np.float32)


if __name__ == "__main__":
    rng = np.random.default_rng(0)
    demo = {"x": rng.standard_normal((B, N_CHANS, T)).astype(np.float32)}
    print("self-test needs full inputs; run test.py instead")
